# revision 2
# baseline (speedup 1.0000x reference)
"""BertAttention (cross-attention, eval) on 8 Trainium2 NeuronCores.

Problem: B=4, SQ=SK=2048, HID=1024, NH=16, HD=64.
  q = hidden @ Wq + bq ; k = ctx @ Wk + bk ; v = ctx @ Wv + bv
  out = softmax(q k^T / 8) v        (per head), heads re-merged.

Sharding (no collectives needed): 8 cores = 4 batches x 2 head-groups.
Core c handles batch b = c//2 and heads hs..hs+8 where hs = (c%2)*8.

Math (all exact): softmax row-shift-invariance cancels the q@bk^T and
bq@bk^T terms; the surviving rank-1 term rT[k] = bq.K[k,:]/8 enters exp()
as a per-partition bias (identically zero for bq=0, in which case the rT
path is compiled out).  exp() is applied without max subtraction (scores
~ N(0,1), safe in f32).  P@V is computed unnormalized with a ones-column
appended to V so PSUM row 64 accumulates the softmax denominator;
reciprocal + broadcast multiply normalizes, then +bv.

Layouts: scores are built transposed (k on partitions, q free) so exp()
output PT feeds the P@V matmul directly as the moving operand.  QK
matmuls are K=128 (HW: ~330 ns for K=64 vs ~190-222 ns for K=128): the Q
projection writes zero-padded tiles qt0 = [q_h0; 0], qt1 = [0; q_h1] and
both heads contract against the same full kt chunk (zero rows contribute
exactly 0).  The host hands the kernel pre-transposed x^T / c^T in bf16
and re-transposes the [512, 2048] per-core output.

v5 structure (HW-calibrated): the pure attention inner loop measures
~1966 ns/kc (ACT-bound; both engines pipelining cleanly), but interleaved
projection groups inject multi-us bubbles into the 2-buffer PSUM score
ring.  So v5 fully STAGES the work: phase 1 runs ALL projections
(Q, K for all 4 head-pairs, then V) back-to-back (PE-bound, paced by the
column-blocked input DMA), phase 2 runs pure attention.  PSUM: score
tiles 2x[128,1024] + ctx 2x[128,1024] = 8 banks; all four pairs' q/k
tiles are held in SBUF simultaneously.
"""

import numpy as np
import ml_dtypes

import concourse.bass as bass
import concourse.mybir as mybir
import concourse.tile as tile
from concourse import bacc
from concourse.bass_utils import run_bass_kernel_spmd

P = 128
B, SQ, SK, HID, NH = 4, 2048, 2048, 1024, 16
HD = 64
N_CORES = 8
NHC = NH // 2          # heads per core = 8
DW = NHC * HD          # per-core output width = 512
VW = NHC * (HD + 1)    # V block width per k-chunk (64 vals + 1 ones col per head)

_BF = ml_dtypes.bfloat16


def build_nc(sq=SQ, sk=SK, hid=HID, nhc=NHC, reps=1, use_bias=False):
    """Build the single-core Bass program (same program runs SPMD on all 8).

    reps > 1 repeats the whole computation (including DMAs) in one NEFF;
    used only for differential wall-clock timing of the kernel body.
    use_bias=True adds the rT columns (needed only when bq != 0).
    """
    hd = HD
    cc_n = hid // P          # contraction chunks (8)
    kc_n = sk // P           # key chunks (16)
    pairs = nhc // 2
    dw = nhc * hd
    vw = nhc * (hd + 1)
    nvx = nhc if use_bias else 0  # extra wv columns producing rT

    bf = mybir.dt.bfloat16
    f32 = mybir.dt.float32
    Exp = mybir.ActivationFunctionType.Exp
    MULT = mybir.AluOpType.mult

    nc = bacc.Bacc("TRN2", target_bir_lowering=False, debug=False)

    xT = nc.dram_tensor("xT", [hid, sq], bf, kind="ExternalInput").ap()
    cT = nc.dram_tensor("cT", [hid, sk], bf, kind="ExternalInput").ap()
    wq = nc.dram_tensor("wq", [hid, dw], bf, kind="ExternalInput").ap()
    wk = nc.dram_tensor("wk", [hid, dw], bf, kind="ExternalInput").ap()
    wv = nc.dram_tensor("wv", [hid, dw + nvx], bf, kind="ExternalInput").ap()
    bv = nc.dram_tensor("bv", [dw], f32, kind="ExternalInput").ap()
    out = nc.dram_tensor("out", [dw, sq], f32, kind="ExternalOutput").ap()

    n_g = sq // 512          # 512-token projection groups (4)
    q2_n = sq // 1024        # 1024-wide q tiles for attention (2)

    with tile.TileContext(nc) as tc:
        with (
            tc.tile_pool(name="const", bufs=1) as cpool,
            tc.tile_pool(name="qk", bufs=4) as qkpool,
            tc.tile_pool(name="pt", bufs=6) as ptpool,
            tc.tile_pool(name="work", bufs=2) as wpool,
            tc.tile_pool(name="psum", bufs=2, space="PSUM") as pspool,
        ):
            for _rep in range(reps):
                xT_sb = cpool.tile([P, cc_n * sq], bf, name="xT_sb")
                cT_sb = cpool.tile([P, cc_n * sk], bf, name="cT_sb")
                wq_sb = cpool.tile([P, cc_n * dw], bf, name="wq_sb")
                wk_sb = cpool.tile([P, cc_n * dw], bf, name="wk_sb")
                wv_sb = cpool.tile([P, cc_n * (dw + nvx)], bf, name="wv_sb")
                v_sb = cpool.tile([P, kc_n * vw], bf, name="v_sb")
                bv_sb = cpool.tile([hd, nhc], f32, name="bv_sb")
                rt_sb = (cpool.tile([P, kc_n * nhc], f32, name="rt_sb")
                         if use_bias else None)

                # qt0/qt1/kt for all four pairs are alive through attention
                qts = []
                for pp in range(pairs):
                    qt0 = qkpool.tile([P, sq], bf, tag="qt0", name=f"qt0_{pp}")
                    qt1 = qkpool.tile([P, sq], bf, tag="qt1", name=f"qt1_{pp}")
                    kt = qkpool.tile([P, sk], bf, tag="kt", name=f"kt{pp}")
                    nc.vector.memset(qt0[64:128, :], 0.0)
                    nc.vector.memset(qt1[0:64, :], 0.0)
                    qts.append((qt0, qt1, kt))

                # ---- input DMA, column-blocked, in first-consumer order ----
                def dma_w(dst, src, width):
                    for cc in range(cc_n):
                        nc.sync.dma_start(dst[:, cc * width:(cc + 1) * width],
                                          src[cc * P:(cc + 1) * P, :])

                def dma_col(dst, src, width, g):
                    for cc in range(cc_n):
                        nc.sync.dma_start(
                            dst[:, cc * width + g * 512:
                                cc * width + (g + 1) * 512],
                            src[cc * P:(cc + 1) * P, g * 512:(g + 1) * 512])

                dma_w(wq_sb, wq, dw)
                for g in range(n_g):
                    dma_col(xT_sb, xT, sq, g)
                dma_w(wk_sb, wk, dw)
                for g in range(n_g):
                    dma_col(cT_sb, cT, sk, g)
                dma_w(wv_sb, wv, dw + nvx)
                nc.sync.dma_start(bv_sb[:, :], bv.rearrange("(h d) -> d h", d=hd))

                # every 65th column of v_sb is a ones column (denominator
                # trick); only those 16 columns need initializing (the
                # V-projection copies fill the rest), and the narrow memset
                # keeps the v_sb WAR window tiny so the next rep's V phase
                # can overlap this rep's attention tail.
                ones_cols = v_sb.rearrange("p (c h w) -> p c h w",
                                           c=kc_n, h=nhc)[:, :, :, hd:hd + 1]
                nc.vector.memset(ones_cols, 1.0)

                # ---- phase 1: all projections -------------------------------
                # Q then K, ordered by token group then pair so each group is
                # usable as soon as its xT/cT column block lands.
                for g in range(n_g):
                    t0 = g * 512
                    for pp in range(pairs):
                        qt0, qt1, kt = qts[pp]
                        q_ps = pspool.tile([P, 1024], f32, tag="st", name="q_ps")
                        for cc in range(cc_n):
                            nc.tensor.matmul(
                                q_ps[:, 0:512],
                                lhsT=wq_sb[:, cc * dw + pp * P:
                                           cc * dw + pp * P + P],
                                rhs=xT_sb[:, cc * sq + t0: cc * sq + t0 + 512],
                                start=(cc == 0), stop=(cc == cc_n - 1))
                        nc.vector.tensor_copy(qt0[0:64, t0:t0 + 512],
                                              q_ps[0:64, 0:512])
                        nc.vector.tensor_copy(qt1[64:128, t0:t0 + 512],
                                              q_ps[64:128, 0:512])
                for g in range(n_g):
                    t0 = g * 512
                    for pp in range(pairs):
                        qt0, qt1, kt = qts[pp]
                        k_ps = pspool.tile([P, 1024], f32, tag="st", name="k_ps")
                        for cc in range(cc_n):
                            nc.tensor.matmul(
                                k_ps[:, 0:512],
                                lhsT=wk_sb[:, cc * dw + pp * P:
                                           cc * dw + pp * P + P],
                                rhs=cT_sb[:, cc * sk + t0: cc * sk + t0 + 512],
                                start=(cc == 0), stop=(cc == cc_n - 1))
                        nc.vector.tensor_copy(kt[:, t0:t0 + 512], k_ps[:, 0:512])
                # V projection (all heads at once) [+ rT columns]
                for kc in range(kc_n):
                    pv_ps = pspool.tile([P, 1024], f32, tag="st", name="pv_ps")
                    for cc in range(cc_n):
                        lhs = cT_sb[:, cc * sk + kc * P: cc * sk + kc * P + P]
                        nc.tensor.matmul(
                            pv_ps[:, 0:dw], lhsT=lhs,
                            rhs=wv_sb[:, cc * (dw + nvx): cc * (dw + nvx) + dw],
                            start=(cc == 0), stop=(cc == cc_n - 1))
                        if use_bias:
                            # rT columns at col 512 = the slot's second bank,
                            # so their accumulation group never shares a bank
                            # with the V group.
                            nc.tensor.matmul(
                                pv_ps[:, 512:512 + nhc], lhsT=lhs,
                                rhs=wv_sb[:, cc * (dw + nvx) + dw:
                                          (cc + 1) * (dw + nvx)],
                                start=(cc == 0), stop=(cc == cc_n - 1))
                    vdst = v_sb[:, kc * vw:(kc + 1) * vw].rearrange(
                        "p (h w) -> p h w", h=nhc)[:, :, 0:hd]
                    vsrc = pv_ps[:, 0:dw].rearrange("p (h w) -> p h w", h=nhc)
                    nc.vector.tensor_copy(vdst, vsrc)
                    if use_bias:
                        nc.vector.tensor_copy(rt_sb[:, kc * nhc:(kc + 1) * nhc],
                                              pv_ps[:, 512:512 + nhc])

                # ---- phase 2: pure attention --------------------------------
                def emit_pv(p, kc, pt0, pt1, ctx0, ctx1):
                    h0, h1 = 2 * p, 2 * p + 1
                    for qh in range(2):
                        cs = slice(qh * 512, (qh + 1) * 512)
                        nc.tensor.matmul(
                            ctx0[0:hd + 1, cs],
                            lhsT=v_sb[:, kc * vw + h0 * (hd + 1):
                                      kc * vw + (h0 + 1) * (hd + 1)],
                            rhs=pt0[:, cs],
                            start=(kc == 0), stop=(kc == kc_n - 1))
                    for qh in range(2):
                        cs = slice(qh * 512, (qh + 1) * 512)
                        nc.tensor.matmul(
                            ctx1[0:hd + 1, cs],
                            lhsT=v_sb[:, kc * vw + h1 * (hd + 1):
                                      kc * vw + (h1 + 1) * (hd + 1)],
                            rhs=pt1[:, cs],
                            start=(kc == 0), stop=(kc == kc_n - 1))

                def emit_finish(p, q2, ctx0, ctx1):
                    # Copy-first: evacuate ctx PSUM immediately so the next
                    # block's first PV isn't WAR-blocked behind the chain.
                    for hh, ctx_ps in ((0, ctx0), (1, ctx1)):
                        h = 2 * p + hh
                        cpy = wpool.tile([hd + 1, 1024], f32, tag="cpy",
                                         name="cpy")
                        nc.vector.tensor_copy(cpy, ctx_ps[0:hd + 1, :])
                        rec = wpool.tile([1, 1024], f32, tag="rec", name="rec")
                        nc.vector.reciprocal(rec, cpy[hd:hd + 1, :])
                        rec_bc = wpool.tile([hd, 1024], f32, tag="recbc",
                                            name="rec_bc")
                        nc.gpsimd.partition_broadcast(rec_bc[:, :], rec[:, :])
                        o_sb = wpool.tile([hd, 1024], f32, tag="osb",
                                          name="o_sb")
                        nc.vector.tensor_tensor(
                            o_sb[:, :], cpy[0:hd, :], rec_bc[:, :], MULT)
                        nc.vector.tensor_scalar_add(o_sb[:, :], o_sb[:, :],
                                                    bv_sb[:, h:h + 1])
                        nc.sync.dma_start(
                            out[p * P + hh * hd: p * P + (hh + 1) * hd,
                                q2 * 1024:(q2 + 1) * 1024],
                            o_sb[:, :])

                carry_pv = None
                carry_fin = None
                for p in range(pairs):
                    h0, h1 = 2 * p, 2 * p + 1
                    qt0_sb, qt1_sb, kt_sb = qts[p]
                    for q2 in range(q2_n):
                        ctx0 = pspool.tile([P, 1024], f32, tag="ctx", name="ctx0")
                        ctx1 = pspool.tile([P, 1024], f32, tag="ctx", name="ctx1")
                        prev = None
                        for kc in range(kc_n):
                            st0 = pspool.tile([P, 1024], f32, tag="st", name="st0")
                            st1 = pspool.tile([P, 1024], f32, tag="st", name="st1")
                            kt_lhs = kt_sb[:, kc * P:(kc + 1) * P]
                            for qh in range(2):
                                qs = q2 * 1024 + qh * 512
                                nc.tensor.matmul(
                                    st0[:, qh * 512:(qh + 1) * 512],
                                    lhsT=kt_lhs,
                                    rhs=qt0_sb[:, qs:qs + 512],
                                    start=True, stop=True)
                            for qh in range(2):
                                qs = q2 * 1024 + qh * 512
                                nc.tensor.matmul(
                                    st1[:, qh * 512:(qh + 1) * 512],
                                    lhsT=kt_lhs,
                                    rhs=qt1_sb[:, qs:qs + 512],
                                    start=True, stop=True)
                            if kc == 0 and carry_pv is not None:
                                emit_pv(*carry_pv)
                                carry_pv = None
                                emit_finish(*carry_fin)
                                carry_fin = None
                            pt0 = ptpool.tile([P, 1024], bf, tag="pt", name="pt0")
                            pt1 = ptpool.tile([P, 1024], bf, tag="pt", name="pt1")
                            if use_bias:
                                nc.scalar.activation(
                                    pt0, st0, Exp,
                                    bias=rt_sb[:, kc * nhc + h0:
                                               kc * nhc + h0 + 1])
                                nc.scalar.activation(
                                    pt1, st1, Exp,
                                    bias=rt_sb[:, kc * nhc + h1:
                                               kc * nhc + h1 + 1])
                            else:
                                nc.scalar.activation(pt0, st0, Exp)
                                nc.scalar.activation(pt1, st1, Exp)
                            if prev is not None:
                                emit_pv(p, prev[0], prev[1], prev[2],
                                        ctx0, ctx1)
                            prev = (kc, pt0, pt1)
                        carry_pv = (p, prev[0], prev[1], prev[2], ctx0, ctx1)
                        carry_fin = (p, q2, ctx0, ctx1)

                emit_pv(*carry_pv)
                emit_finish(*carry_fin)

    nc.compile()
    return nc


_NC_CACHE = {}


def _get_nc(use_bias):
    key = ("nc", use_bias)
    if key not in _NC_CACHE:
        _NC_CACHE[key] = build_nc(use_bias=use_bias)
    return _NC_CACHE[key]


def _prep_core_inputs(hidden_states, context, Wq, bq, Wk, bk, Wv, bv,
                      use_bias=None):
    """Host-side shard + layout prep. Returns list of 8 in_maps."""
    if use_bias is None:
        use_bias = bool(np.any(np.asarray(bq) != 0))
    _prep_core_inputs.use_bias = use_bias
    scale = 1.0 / np.sqrt(HD)
    xT_b = []
    cT_b = []
    for b in range(B):
        xT_b.append(np.ascontiguousarray(hidden_states[b].T).astype(_BF))
        cT_b.append(np.ascontiguousarray(context[b].T).astype(_BF))
    in_maps = []
    for c in range(N_CORES):
        b = c // 2
        hs = (c % 2) * NHC
        cols = slice(hs * HD, (hs + NHC) * HD)
        wq_c = (Wq[:, cols] * scale).astype(_BF)
        wk_c = Wk[:, cols].astype(_BF)
        if use_bias:
            # rT producer columns: (Wk_h @ bq_h) * scale  for each head h
            wkr = np.empty((HID, NHC), np.float32)
            for h in range(NHC):
                hcols = slice((hs + h) * HD, (hs + h + 1) * HD)
                wkr[:, h] = (Wk[:, hcols] @ bq[hcols]) * scale
            wv_c = np.concatenate(
                [Wv[:, cols].astype(np.float32), wkr], axis=1).astype(_BF)
        else:
            wv_c = Wv[:, cols].astype(_BF)
        in_maps.append({
            "xT": xT_b[b],
            "cT": cT_b[b],
            "wq": np.ascontiguousarray(wq_c),
            "wk": np.ascontiguousarray(wk_c),
            "wv": np.ascontiguousarray(wv_c),
            "bv": np.ascontiguousarray(bv[cols]).astype(np.float32),
        })
    return in_maps


def kernel(hidden_states, context, Wq, bq, Wk, bk, Wv, bv):
    hidden_states = np.asarray(hidden_states, dtype=np.float32)
    context = np.asarray(context, dtype=np.float32)
    Wq = np.asarray(Wq, dtype=np.float32)
    bq = np.asarray(bq, dtype=np.float32)
    Wk = np.asarray(Wk, dtype=np.float32)
    bk = np.asarray(bk, dtype=np.float32)
    Wv = np.asarray(Wv, dtype=np.float32)
    bv = np.asarray(bv, dtype=np.float32)

    in_maps = _prep_core_inputs(
        hidden_states, context, Wq, bq, Wk, bk, Wv, bv)
    nc = _get_nc(_prep_core_inputs.use_bias)
    res = run_bass_kernel_spmd(nc, in_maps, list(range(N_CORES)))
    full = np.empty((B, SQ, NH * HD), np.float32)
    for c in range(N_CORES):
        b = c // 2
        hs = (c % 2) * NHC
        cols = slice(hs * HD, (hs + NHC) * HD)
        full[b, :, cols] = res.results[c]["out"].T
    return full
